# revision 18
# baseline (speedup 1.0000x reference)
"""Causal self-attention TRN2 kernel, 8-core SPMD (batch x head-group sharded).

Model: B=4, T=2048, D=1024, H=16 heads x 64. out = softmax(mask(QK^T/8)) V W_proj^T.

Sharding: core c handles batch c//2 and head-group g = c%2 (heads 8g..8g+7,
i.e. 4 head-pairs). Each core computes Q/K/V projections only for its own 8
heads, runs full causal attention for those heads over all 2048 tokens, and
emits the partial output projection over its 512 head-dims as [D, T] fp32.
The host sums the two partial projections per batch while gathering (the
tensor-parallel reduce done on unshard) -- no device collectives.

Per-core PE work: QKV 196608 + QK 139264 + PV 139264 + proj 65536 = 540672
cycles @ 2.4 GHz ~= 225 us, vs 688k for a token-sharded variant that
recomputes K/V for the whole batch on both cores of a pair.

The build merges two instruction streams on the PE: the attention stream
(QK -> exp -> PV per k-tile) and a "filler" stream of dense projection
chunks (K/Q/V, then the output projection of the previous q-chunk), paced
~1:1 in PE cycles. Per k-tile the QK+PV time exactly equals the ACT
engine's exp time, so without filler the PE would run in lockstep with ACT
and stall on any hiccup; the filler gives ACT slack and keeps the HAM
clock warm.

All matmuls bf16 (fp32 accumulate); scores are pre-scaled by 1/8 via the Q
weights and |score| <= ~3.1, so softmax needs no max subtraction.
"""

import os
from contextlib import ExitStack

import numpy as np
import ml_dtypes

import concourse.bass as bass
import concourse.mybir as mybir
import concourse.tile as tile
from concourse import bacc
from concourse.bass_utils import run_bass_kernel_spmd

BF16 = mybir.dt.bfloat16
F32 = mybir.dt.float32
EXP = mybir.ActivationFunctionType.Exp

B, T, D = 4, 2048, 1024
H, DH = 16, 64
NCORES = 8
NP = 4           # head pairs per core (8 heads)
KT = 16          # 128-token k-tiles per batch
NJ = 4           # 512-query chunks

_cached = {}

if os.environ.get("BASS_LDW_OPT", "") == "1":
    # A/B experiment: let walrus keep/overlap LDWEIGHTS (default path passes
    # --enable-ldw-opt=false)
    from concourse import bass_utils as _bu
    _orig_run_command = _bu.run_command
    def _patched_run_command(argv, **kwargs):
        argv = [a.replace("--enable-ldw-opt=false", "--enable-ldw-opt=true")
                if isinstance(a, str) else a for a in argv]
        return _orig_run_command(argv, **kwargs)
    _bu.run_command = _patched_run_command


def _build_program():
    nc = bacc.Bacc("TRN2", name="causal_attn_tp")

    xTd = nc.dram_tensor("xT", [D, T], BF16, kind="ExternalInput")
    w_qT = nc.dram_tensor("w_qT", [D, 512], BF16, kind="ExternalInput")
    w_kT = nc.dram_tensor("w_kT", [D, 512], BF16, kind="ExternalInput")
    w_vT = nc.dram_tensor("w_vT", [D, 512], BF16, kind="ExternalInput")
    w_pT = nc.dram_tensor("w_pT", [512, D], BF16, kind="ExternalInput")
    maskd = nc.dram_tensor("mask", [128, 128], BF16, kind="ExternalInput")
    outd = nc.dram_tensor("out_T", [D, T], F32, kind="ExternalOutput")

    with ExitStack() as ctx:
        tc = ctx.enter_context(tile.TileContext(nc))

        const = ctx.enter_context(tc.tile_pool(name="const", bufs=1))
        xp = ctx.enter_context(tc.tile_pool(name="xsb", bufs=1))
        wqp = ctx.enter_context(tc.tile_pool(name="wqsb", bufs=1))
        wkp = ctx.enter_context(tc.tile_pool(name="wksb", bufs=1))
        wvp = ctx.enter_context(tc.tile_pool(name="wvsb", bufs=1))
        wpp = ctx.enter_context(tc.tile_pool(name="wpsb", bufs=1))
        kp = ctx.enter_context(tc.tile_pool(name="ksb", bufs=1))
        qp = ctx.enter_context(tc.tile_pool(name="qsb", bufs=1))
        vpool = ctx.enter_context(tc.tile_pool(name="vsb", bufs=1))
        opool = ctx.enter_context(tc.tile_pool(name="osb", bufs=2))
        ppool = ctx.enter_context(tc.tile_pool(name="pex", bufs=6))
        ovpool = ctx.enter_context(tc.tile_pool(name="ovsb", bufs=2))
        brpool = ctx.enter_context(tc.tile_pool(name="brsb", bufs=2))
        outsb = ctx.enter_context(tc.tile_pool(name="outsb", bufs=4))
        mm_ps = ctx.enter_context(tc.tile_pool(name="mm_ps", bufs=2, space="PSUM"))
        st_ps = ctx.enter_context(tc.tile_pool(name="st_ps", bufs=2, space="PSUM"))

        mask_sb = const.tile([128, 128], BF16)
        ones_sb = const.tile([65, 64], F32, name="ones_sb")
        nc.vector.memset(ones_sb[:, :], 1.0)
        xkv = [xp.tile([128, T], BF16, tag=f"x{d}", name=f"x{d}") for d in range(8)]
        wq = [wqp.tile([128, 512], BF16, tag=f"wq{d}", name=f"wq{d}") for d in range(8)]
        wk = [wkp.tile([128, 512], BF16, tag=f"wk{d}", name=f"wk{d}") for d in range(8)]
        wv = [wvp.tile([128, 512], BF16, tag=f"wv{d}", name=f"wv{d}") for d in range(8)]
        wp = [wpp.tile([128, D], BF16, tag=f"wp{p}", name=f"wp{p}") for p in range(NP)]
        K_sb = [kp.tile([128, T], BF16, tag=f"k{p}", name=f"k{p}") for p in range(NP)]
        Q_sb = [qp.tile([128, T], BF16, tag=f"q{p}", name=f"q{p}") for p in range(NP)]
        V_sb = [vpool.tile([128, 8, DH + 1], BF16, tag=f"v{m}", name=f"v{m}")
                for m in range(KT)]

        # ---- input DMAs; issue order == arrival order per queue, matched to
        # the prologue's consumption order (K chunks, then V tiles, then Q):
        #   sync:   x c0 even-d, wv, x c1-3 even-d
        #   gpsimd: x odd-d (c-major)
        #   scalar: mask, wk, wq, wp
        def dma_x(eng, d, c):
            eng.dma_start(out=xkv[d][:, 512 * c:512 * c + 512],
                          in_=xTd[128 * d:128 * d + 128, 512 * c:512 * c + 512])

        nc.scalar.dma_start(out=mask_sb[:, :], in_=maskd[:, :])
        for d in range(0, 8, 2):
            dma_x(nc.sync, d, 0)
        for c in range(4):
            for d in range(1, 8, 2):
                dma_x(nc.gpsimd, d, c)
        for d in range(8):
            nc.sync.dma_start(out=wv[d][:, :], in_=w_vT[128 * d:128 * d + 128, :])
        for d in range(0, 8, 2):
            nc.sync.dma_start(out=wq[d][:, :], in_=w_qT[128 * d:128 * d + 128, :])
        for c in range(1, 4):
            for d in range(0, 8, 2):
                dma_x(nc.sync, d, c)
        for d in range(8):
            nc.scalar.dma_start(out=wk[d][:, :], in_=w_kT[128 * d:128 * d + 128, :])
        for d in range(1, 8, 2):
            nc.scalar.dma_start(out=wq[d][:, :], in_=w_qT[128 * d:128 * d + 128, :])
        for p in range(NP):
            nc.scalar.dma_start(out=wp[p][:, :], in_=w_pT[128 * p:128 * p + 128, :])

        # ---- filler-stream emitters (dense full-array projection chunks) ----
        def emit_kchunk(p, n):
            ps = mm_ps.tile([128, 512], F32, tag="ps", name="ps")
            for d in range(8):
                nc.tensor.matmul(ps[:, :],
                                 lhsT=wk[d][:, 128 * p:128 * p + 128],
                                 rhs=xkv[d][:, 512 * n:512 * n + 512],
                                 start=(d == 0), stop=(d == 7))
            nc.vector.tensor_copy(K_sb[p][:, 512 * n:512 * n + 512], ps[:, :])

        def emit_qchunk(p, n):
            ps = mm_ps.tile([128, 512], F32, tag="ps", name="ps")
            for d in range(8):
                nc.tensor.matmul(ps[:, :],
                                 lhsT=wq[d][:, 128 * p:128 * p + 128],
                                 rhs=xkv[d][:, 512 * n:512 * n + 512],
                                 start=(d == 0), stop=(d == 7))
            nc.vector.tensor_copy(Q_sb[p][:, 512 * n:512 * n + 512], ps[:, :])

        def emit_v(m):
            ps = mm_ps.tile([128, 512], F32, tag="ps", name="ps")
            for d in range(8):
                nc.tensor.matmul(ps[:, :],
                                 lhsT=xkv[d][:, 128 * m:128 * m + 128],
                                 rhs=wv[d][:, :],
                                 start=(d == 0), stop=(d == 7))
            nc.vector.tensor_copy(V_sb[m][:, 0:8, 0:DH],
                                  ps[:, :].rearrange("p (h e) -> p h e", h=8))
            nc.vector.memset(V_sb[m][:, :, DH:DH + 1], 1.0)

        O_tiles = {}

        def emit_proj(J, m):
            ps = mm_ps.tile([128, 512], F32, tag="ps", name="ps")
            for pp in range(NP):
                nc.tensor.matmul(ps[:, :],
                                 lhsT=wp[pp][:, 128 * m:128 * m + 128],
                                 rhs=O_tiles[(J, pp)][:, :],
                                 start=(pp == 0), stop=(pp == NP - 1))
            ob = outsb.tile([128, 512], F32, tag="ob", name="ob")
            nc.vector.tensor_copy(ob[:, :], ps[:, :])
            nc.sync.dma_start(out=outd[128 * m:128 * m + 128, 512 * J:512 * J + 512],
                              in_=ob[:, :])

        # ---- filler streams: a static K/Q/V queue with dependency markers,
        # plus a dynamic deque of output-projection chunks (producible only
        # after a q-chunk's attention completes). pace() interleaves them
        # round-robin into the PE stream at ~1:1 cycles vs attention. ----
        bq = []
        bmark = {}
        for n in range(NJ):
            for p in range(NP):
                bq.append((4096, (lambda p=p, n=n: emit_kchunk(p, n))))
                bmark[("K", p, n)] = len(bq)
            for m in range(4 * n, 4 * n + 4):
                bq.append((4096, (lambda m=m: emit_v(m))))
                bmark[("V", m)] = len(bq)
            for p in range(NP):
                bq.append((4096, (lambda p=p, n=n: emit_qchunk(p, n))))
                bmark[("Q", p, n)] = len(bq)

        # attention PE cycles total: sum over (J, p, ki) of 3*nw — the two
        # heads' QK matmuls run concurrently on PE row groups 0-63/64-127
        # (tile_position from the partition-sliced lhsT), so a QK pair costs
        # nw cycles, plus 2*nw for the two sequential PV matmuls
        TOTAL_A = 12 * (2048 * (0 + 1 + 2 + 3) + 1280 * 4)

        pq = []  # deque of (cost, fn, J)
        mstate = {"bpos": 0, "cB": 0, "cA": 0, "rr": 0,
                  "pairA0": 0, "pairB0": 0, "R": 1.0,
                  "future_proj": 32 * 2048}

        def pop_static():
            cost, fn = bq[mstate["bpos"]]
            mstate["bpos"] += 1
            mstate["cB"] += cost
            fn()

        def pop_proj():
            cost, fn, _ = pq.pop(0)
            mstate["cB"] += cost
            fn()

        def pop_b():
            have_s = mstate["bpos"] < len(bq)
            have_p = bool(pq)
            if have_s and have_p:
                mstate["rr"] ^= 1
                (pop_static if mstate["rr"] else pop_proj)()
            elif have_s:
                pop_static()
            elif have_p:
                pop_proj()

        def force(idx):
            while mstate["bpos"] < idx:
                pop_static()

        def drain_proj_thru(Jmax):
            while pq and pq[0][2] <= Jmax:
                pop_proj()

        def begin_pair():
            # proportional pacing: spread the remaining filler uniformly over
            # the remaining attention so the PE never drops into lockstep
            # with ACT (which lets the HAM clock re-throttle)
            b_left = (sum(c for c, _ in bq[mstate["bpos"]:])
                      + sum(c for c, _, _ in pq) + mstate["future_proj"])
            a_left = max(1, TOTAL_A - mstate["cA"])
            mstate["R"] = min(1.5, b_left / a_left)
            mstate["pairA0"] = mstate["cA"]
            mstate["pairB0"] = mstate["cB"]

        def pace(slack=0):
            while ((mstate["bpos"] < len(bq) or pq)
                   and (mstate["cB"] - mstate["pairB0"])
                       < mstate["R"] * (mstate["cA"] - mstate["pairA0"]) + slack):
                pop_b()

        # softmax normalization: the PV rows 64 hold the per-query exp sums
        # (via the ones column of V). Part a (right after the pair's last PV)
        # drains pv PSUM to SBUF so the next pair's PV can start immediately;
        # part b is deferred into the next pair's stream: broadcast the sums
        # across 64 partitions with a K=1 ones-matmul, fast-reciprocal on DVE,
        # and scale on GpSimd (all-SBUF).
        pending = {"fn": None}

        def normalize_a(pvs):
            # ACT does these copies: it has a natural lull at the pair
            # boundary (the next pair's first exp waits on its QK anyway),
            # while DVE is busy with filler-chunk copies the PE waits on
            ov = ovpool.tile([65, 2, 512], F32, tag="ov", name="ov")
            for hi in (0, 1):
                nc.scalar.copy(ov[0:65, hi, :], pvs[hi][0:65, :])
            return ov

        def normalize_b(J, p, ov):
            brec = brpool.tile([64, 2, 512], F32, tag="brec", name="brec")
            for hi in (0, 1):
                bcm = mm_ps.tile([64, 512], F32, tag="ps", name="bcm")
                nc.tensor.matmul(bcm[:, :], lhsT=ones_sb[64:65, 0:64],
                                 rhs=ov[64:65, hi, :], start=True, stop=True)
                nc.vector.reciprocal_approx_fast(brec[:, hi, :], bcm[0:64, :])
            ot = opool.tile([128, 512], BF16, tag=f"o{p}", name=f"o{p}")
            O_tiles[(J, p)] = ot
            for hi in (0, 1):
                nc.gpsimd.tensor_mul(ot[64 * hi:64 * hi + 64, :],
                                     ov[0:64, hi, :], brec[:, hi, :])

        def flush_norm():
            if pending["fn"] is not None:
                pending["fn"]()
                pending["fn"] = None

        # ---- main loop: q-chunk-major attention with paced filler. PV runs
        # one k-tile behind QK so the PE never waits head-of-line on an exp.
        for J in range(NJ):
            if J >= 2:
                # O tiles are double-buffered per pair: proj of chunk J-2
                # must be emitted before normalize(J) reuses its slot
                drain_proj_thru(J - 2)
            nki = 4 * J + 4

            def emit_pv(pvs, ki, qc0, nw, pb, p, nki):
                force(bmark[("V", ki)])
                for hi in (0, 1):
                    nc.tensor.matmul(
                        pvs[hi][:, qc0:qc0 + nw],
                        lhsT=V_sb[ki][:, 2 * p + hi, :],
                        rhs=pb[:, hi, 0:nw],
                        start=(ki == 0), stop=(ki == nki - 1))
                mstate["cA"] += 2 * nw

            for p in range(NP):
                force(bmark[("K", p, J)])
                force(bmark[("Q", p, J)])
                # also force the NEXT pair's K/Q (and the V tiles between
                # them in the block) now, so their PSUM->SBUF copies are a
                # full pair old by the time that pair's QK LDWEIGHTS needs
                # them — otherwise the PE waits on the DVE queue
                if p + 1 < NP:
                    force(bmark[("Q", p + 1, J)])
                elif J + 1 < NJ:
                    force(bmark[("Q", 0, J + 1)])
                begin_pair()
                pvs = {}
                for hi in (0, 1):
                    pvs[hi] = mm_ps.tile([65, 512], F32, tag="pv", name="pv", bufs=2)
                pend = None
                for ki in range(nki):
                    e = ki - 4 * J
                    qc0 = 0 if e < 0 else 128 * e
                    nw = 512 - qc0
                    st = st_ps.tile([128, 2, 512], F32, tag="st", name="st")
                    for hi, h_off in ((0, 0), (1, 64)):
                        nc.tensor.matmul(
                            st[:, hi, 0:nw],
                            lhsT=K_sb[p][h_off:h_off + 64, 128 * ki:128 * ki + 128],
                            rhs=Q_sb[p][h_off:h_off + 64, 512 * J + qc0:512 * J + 512],
                            start=True, stop=True)
                    mstate["cA"] += nw
                    pb = ppool.tile([128, 2, 512], BF16, tag="pb", name="pb")
                    nc.scalar.activation(pb[:, :, 0:nw], st[:, :, 0:nw], EXP)
                    if e >= 0:
                        # multiplicative 0/1 causal stair on both heads; the
                        # mask operand repeats via a 0-stride dim
                        msrc = mask_sb[:, 0:128]
                        mrep = bass.AP(tensor=msrc.tensor, offset=msrc.offset,
                                       ap=[list(msrc.ap[0]), [0, 2], [1, 128]])
                        nc.vector.tensor_mul(pb[:, :, 0:128], pb[:, :, 0:128], mrep)
                    pace()
                    if ki == 1:
                        flush_norm()
                    if pend is not None:
                        emit_pv(pvs, *pend, p, nki)
                    pend = (ki, qc0, nw, pb)
                emit_pv(pvs, *pend, p, nki)
                ov = normalize_a(pvs)
                pending["fn"] = (lambda J=J, p=p, ov=ov: normalize_b(J, p, ov))
                pace()
            flush_norm()
            for m in range(8):
                pq.append((2048, (lambda J=J, m=m: emit_proj(J, m)), J))
            mstate["future_proj"] = 2048 * 8 * (NJ - 1 - J)

        drain_proj_thru(NJ)
        force(len(bq))

    nc.finalize()
    return nc


def _host_inputs(x, W_qkv, W_proj):
    bf = ml_dtypes.bfloat16
    kk, qq = np.meshgrid(np.arange(128), np.arange(128), indexing="ij")
    stair = np.ascontiguousarray((kk <= qq).astype(np.float32).astype(bf))

    in_maps = []
    for c in range(NCORES):
        b, g = c // 2, c % 2
        xT = np.ascontiguousarray(x[b].T.astype(bf))  # [D, T]
        s0 = 512 * g
        wq_ = np.ascontiguousarray((W_qkv[s0:s0 + 512] / 8.0).T.astype(bf))
        wk_ = np.ascontiguousarray(W_qkv[D + s0:D + s0 + 512].T.astype(bf))
        wv_ = np.ascontiguousarray(W_qkv[2 * D + s0:2 * D + s0 + 512].T.astype(bf))
        wp_ = np.ascontiguousarray(W_proj.T[s0:s0 + 512, :].astype(bf))
        in_maps.append({
            "xT": xT,
            "w_qT": wq_, "w_kT": wk_, "w_vT": wv_, "w_pT": wp_,
            "mask": stair,
        })
    return in_maps


def _run(inputs, trace=False, trace_cores=None):
    if "nc" not in _cached:
        _cached["nc"] = _build_program()
    nc = _cached["nc"]
    in_maps = _host_inputs(inputs["x"], inputs["W_qkv"], inputs["W_proj"])
    res = run_bass_kernel_spmd(
        nc, in_maps, core_ids=list(range(NCORES)),
        trace=trace, trace_cores=trace_cores,
    )
    out = np.zeros((B, T, D), np.float32)
    for b in range(B):
        acc = res.results[2 * b]["out_T"] + res.results[2 * b + 1]["out_T"]
        out[b] = acc.T  # [D, T] -> [T, D]
    return out, res


def kernel(**inputs) -> np.ndarray:
    out, _ = _run(inputs, trace=os.environ.get("KERNEL_TRACE", "") == "1")
    return out


# revision 20
# speedup vs baseline: 1.1183x; 1.1183x over previous
"""Causal self-attention TRN2 kernel, 8-core SPMD (batch x head-group sharded).

Model: B=4, T=2048, D=1024, H=16 heads x 64. out = softmax(mask(QK^T/8)) V W_proj^T.

Sharding: core c handles batch c//2 and head-group g = c%2 (heads 8g..8g+7,
i.e. 4 head-pairs). Each core computes Q/K/V projections only for its own 8
heads, runs full causal attention for those heads over all 2048 tokens, and
emits the partial output projection over its 512 head-dims as [D, T] fp32.
The host sums the two partial projections per batch while gathering (the
tensor-parallel reduce done on unshard) -- no device collectives.

Per-core PE work: QKV 196608 + QK 139264 + PV 139264 + proj 65536 = 540672
cycles @ 2.4 GHz ~= 225 us, vs 688k for a token-sharded variant that
recomputes K/V for the whole batch on both cores of a pair.

The build merges two instruction streams on the PE: the attention stream
(QK -> exp -> PV per k-tile) and a "filler" stream of dense projection
chunks (K/Q/V, then the output projection of the previous q-chunk), paced
~1:1 in PE cycles. Per k-tile the QK+PV time exactly equals the ACT
engine's exp time, so without filler the PE would run in lockstep with ACT
and stall on any hiccup; the filler gives ACT slack and keeps the HAM
clock warm.

All matmuls bf16 (fp32 accumulate); scores are pre-scaled by 1/8 via the Q
weights and |score| <= ~3.1, so softmax needs no max subtraction.
"""

import os
from contextlib import ExitStack

import numpy as np
import ml_dtypes

import concourse.bass as bass
import concourse.mybir as mybir
import concourse.tile as tile
from concourse import bacc
from concourse.bass_utils import run_bass_kernel_spmd

BF16 = mybir.dt.bfloat16
F32 = mybir.dt.float32
EXP = mybir.ActivationFunctionType.Exp

B, T, D = 4, 2048, 1024
H, DH = 16, 64
NCORES = 8
NP = 4           # head pairs per core (8 heads)
KT = 16          # 128-token k-tiles per batch
NJ = 4           # 512-query chunks

_cached = {}

if os.environ.get("BASS_LDW_OPT", "") == "1":
    # A/B experiment: let walrus keep/overlap LDWEIGHTS (default path passes
    # --enable-ldw-opt=false)
    from concourse import bass_utils as _bu
    _orig_run_command = _bu.run_command
    def _patched_run_command(argv, **kwargs):
        argv = [a.replace("--enable-ldw-opt=false", "--enable-ldw-opt=true")
                if isinstance(a, str) else a for a in argv]
        return _orig_run_command(argv, **kwargs)
    _bu.run_command = _patched_run_command


def _build_program():
    nc = bacc.Bacc("TRN2", name="causal_attn_tp")

    xTd = nc.dram_tensor("xT", [D, T], BF16, kind="ExternalInput")
    w_qT = nc.dram_tensor("w_qT", [D, 512], BF16, kind="ExternalInput")
    w_kT = nc.dram_tensor("w_kT", [D, 512], BF16, kind="ExternalInput")
    w_vT = nc.dram_tensor("w_vT", [D, 512], BF16, kind="ExternalInput")
    w_pT = nc.dram_tensor("w_pT", [512, D], BF16, kind="ExternalInput")
    maskd = nc.dram_tensor("mask", [128, 128], BF16, kind="ExternalInput")
    outd = nc.dram_tensor("out_T", [D, T], F32, kind="ExternalOutput")

    with ExitStack() as ctx:
        tc = ctx.enter_context(tile.TileContext(nc))

        const = ctx.enter_context(tc.tile_pool(name="const", bufs=1))
        xp = ctx.enter_context(tc.tile_pool(name="xsb", bufs=1))
        wqp = ctx.enter_context(tc.tile_pool(name="wqsb", bufs=1))
        wkp = ctx.enter_context(tc.tile_pool(name="wksb", bufs=1))
        wvp = ctx.enter_context(tc.tile_pool(name="wvsb", bufs=1))
        wpp = ctx.enter_context(tc.tile_pool(name="wpsb", bufs=1))
        kp = ctx.enter_context(tc.tile_pool(name="ksb", bufs=1))
        qp = ctx.enter_context(tc.tile_pool(name="qsb", bufs=1))
        vpool = ctx.enter_context(tc.tile_pool(name="vsb", bufs=1))
        opool = ctx.enter_context(tc.tile_pool(name="osb", bufs=2))
        ppool = ctx.enter_context(tc.tile_pool(name="pex", bufs=6))
        ovpool = ctx.enter_context(tc.tile_pool(name="ovsb", bufs=2))
        brpool = ctx.enter_context(tc.tile_pool(name="brsb", bufs=2))
        outsb = ctx.enter_context(tc.tile_pool(name="outsb", bufs=4))
        mm_ps = ctx.enter_context(tc.tile_pool(name="mm_ps", bufs=2, space="PSUM"))
        st_ps = ctx.enter_context(tc.tile_pool(name="st_ps", bufs=2, space="PSUM"))

        mask_sb = const.tile([128, 128], BF16)
        ones_sb = const.tile([65, 64], BF16, name="ones_sb")
        nc.vector.memset(ones_sb[:, :], 1.0)
        xkv = [xp.tile([128, T], BF16, tag=f"x{d}", name=f"x{d}") for d in range(8)]
        wq = [wqp.tile([128, 512], BF16, tag=f"wq{d}", name=f"wq{d}") for d in range(8)]
        wk = [wkp.tile([128, 512], BF16, tag=f"wk{d}", name=f"wk{d}") for d in range(8)]
        wv = [wvp.tile([128, 512], BF16, tag=f"wv{d}", name=f"wv{d}") for d in range(8)]
        wp = [wpp.tile([128, D], BF16, tag=f"wp{p}", name=f"wp{p}") for p in range(NP)]
        K_sb = [kp.tile([128, T], BF16, tag=f"k{p}", name=f"k{p}") for p in range(NP)]
        Q_sb = [qp.tile([128, T], BF16, tag=f"q{p}", name=f"q{p}") for p in range(NP)]
        V_sb = [vpool.tile([128, 8, DH + 1], BF16, tag=f"v{m}", name=f"v{m}")
                for m in range(KT)]

        # ---- input DMAs; issue order == arrival order per queue, matched to
        # the prologue's consumption order (K chunks, then V tiles, then Q):
        #   sync:   x c0 even-d, wv, x c1-3 even-d
        #   gpsimd: x odd-d (c-major)
        #   scalar: mask, wk, wq, wp
        def dma_x(eng, d, c):
            eng.dma_start(out=xkv[d][:, 512 * c:512 * c + 512],
                          in_=xTd[128 * d:128 * d + 128, 512 * c:512 * c + 512])

        nc.scalar.dma_start(out=mask_sb[:, :], in_=maskd[:, :])
        for d in range(0, 8, 2):
            dma_x(nc.sync, d, 0)
        for c in range(4):
            for d in range(1, 8, 2):
                dma_x(nc.gpsimd, d, c)
        for d in range(8):
            nc.sync.dma_start(out=wv[d][:, :], in_=w_vT[128 * d:128 * d + 128, :])
        for d in range(0, 8, 2):
            nc.sync.dma_start(out=wq[d][:, :], in_=w_qT[128 * d:128 * d + 128, :])
        for c in range(1, 4):
            for d in range(0, 8, 2):
                dma_x(nc.sync, d, c)
        for d in range(8):
            nc.scalar.dma_start(out=wk[d][:, :], in_=w_kT[128 * d:128 * d + 128, :])
        for d in range(1, 8, 2):
            nc.scalar.dma_start(out=wq[d][:, :], in_=w_qT[128 * d:128 * d + 128, :])
        for p in range(NP):
            nc.scalar.dma_start(out=wp[p][:, :], in_=w_pT[128 * p:128 * p + 128, :])

        # ---- filler-stream emitters (dense full-array projection chunks) ----
        def emit_kchunk(p, n):
            ps = mm_ps.tile([128, 512], F32, tag="ps", name="ps")
            for d in range(8):
                nc.tensor.matmul(ps[:, :],
                                 lhsT=wk[d][:, 128 * p:128 * p + 128],
                                 rhs=xkv[d][:, 512 * n:512 * n + 512],
                                 start=(d == 0), stop=(d == 7))
            nc.vector.tensor_copy(K_sb[p][:, 512 * n:512 * n + 512], ps[:, :])

        def emit_qchunk(p, n):
            ps = mm_ps.tile([128, 512], F32, tag="ps", name="ps")
            for d in range(8):
                nc.tensor.matmul(ps[:, :],
                                 lhsT=wq[d][:, 128 * p:128 * p + 128],
                                 rhs=xkv[d][:, 512 * n:512 * n + 512],
                                 start=(d == 0), stop=(d == 7))
            nc.vector.tensor_copy(Q_sb[p][:, 512 * n:512 * n + 512], ps[:, :])

        def emit_v(m):
            ps = mm_ps.tile([128, 512], F32, tag="ps", name="ps")
            for d in range(8):
                nc.tensor.matmul(ps[:, :],
                                 lhsT=xkv[d][:, 128 * m:128 * m + 128],
                                 rhs=wv[d][:, :],
                                 start=(d == 0), stop=(d == 7))
            nc.vector.tensor_copy(V_sb[m][:, 0:8, 0:DH],
                                  ps[:, :].rearrange("p (h e) -> p h e", h=8))
            nc.vector.memset(V_sb[m][:, :, DH:DH + 1], 1.0)

        O_tiles = {}

        def emit_proj(J, m):
            ps = mm_ps.tile([128, 512], F32, tag="ps", name="ps")
            for pp in range(NP):
                nc.tensor.matmul(ps[:, :],
                                 lhsT=wp[pp][:, 128 * m:128 * m + 128],
                                 rhs=O_tiles[(J, pp)][:, :],
                                 start=(pp == 0), stop=(pp == NP - 1))
            ob = outsb.tile([128, 512], F32, tag="ob", name="ob")
            nc.vector.tensor_copy(ob[:, :], ps[:, :])
            nc.sync.dma_start(out=outd[128 * m:128 * m + 128, 512 * J:512 * J + 512],
                              in_=ob[:, :])

        # ---- filler streams: a static K/Q/V queue with dependency markers,
        # plus a dynamic deque of output-projection chunks (producible only
        # after a q-chunk's attention completes). pace() interleaves them
        # round-robin into the PE stream at ~1:1 cycles vs attention. ----
        bq = []
        bmark = {}
        for n in range(NJ):
            for p in range(NP):
                bq.append((4096, (lambda p=p, n=n: emit_kchunk(p, n))))
                bmark[("K", p, n)] = len(bq)
            for m in range(4 * n, 4 * n + 4):
                bq.append((4096, (lambda m=m: emit_v(m))))
                bmark[("V", m)] = len(bq)
            for p in range(NP):
                bq.append((4096, (lambda p=p, n=n: emit_qchunk(p, n))))
                bmark[("Q", p, n)] = len(bq)

        # attention PE cycles total: sum over (J, p, ki) of 3*nw — the two
        # heads' QK matmuls run concurrently on PE row groups 0-63/64-127
        # (tile_position from the partition-sliced lhsT), so a QK pair costs
        # nw cycles, plus 2*nw for the two sequential PV matmuls
        TOTAL_A = 12 * (2048 * (0 + 1 + 2 + 3) + 1280 * 4)

        pq = []  # deque of (cost, fn, J)
        mstate = {"bpos": 0, "cB": 0, "cA": 0, "rr": 0,
                  "pairA0": 0, "pairB0": 0, "R": 1.0,
                  "future_proj": 32 * 2048}

        def pop_static():
            cost, fn = bq[mstate["bpos"]]
            mstate["bpos"] += 1
            mstate["cB"] += cost
            fn()

        def pop_proj():
            cost, fn, _ = pq.pop(0)
            mstate["cB"] += cost
            fn()

        def pop_b():
            have_s = mstate["bpos"] < len(bq)
            have_p = bool(pq)
            if have_s and have_p:
                mstate["rr"] ^= 1
                (pop_static if mstate["rr"] else pop_proj)()
            elif have_s:
                pop_static()
            elif have_p:
                pop_proj()

        def force(idx):
            while mstate["bpos"] < idx:
                pop_static()

        def drain_proj_thru(Jmax):
            while pq and pq[0][2] <= Jmax:
                pop_proj()

        def begin_pair():
            # proportional pacing: spread the remaining filler uniformly over
            # the remaining attention so the PE never drops into lockstep
            # with ACT (which lets the HAM clock re-throttle)
            b_left = (sum(c for c, _ in bq[mstate["bpos"]:])
                      + sum(c for c, _, _ in pq) + mstate["future_proj"])
            a_left = max(1, TOTAL_A - mstate["cA"])
            mstate["R"] = min(1.5, b_left / a_left)
            mstate["pairA0"] = mstate["cA"]
            mstate["pairB0"] = mstate["cB"]

        def pace(slack=0):
            while ((mstate["bpos"] < len(bq) or pq)
                   and (mstate["cB"] - mstate["pairB0"])
                       < mstate["R"] * (mstate["cA"] - mstate["pairA0"]) + slack):
                pop_b()

        # softmax normalization: the PV rows 64 hold the per-query exp sums
        # (via the ones column of V). Part a (right after the pair's last PV)
        # drains pv PSUM to SBUF so the next pair's PV can start immediately;
        # part b is deferred into the next pair's stream: broadcast the sums
        # across 64 partitions with a K=1 ones-matmul, fast-reciprocal on DVE,
        # and scale on GpSimd (all-SBUF).
        pending = {"fn": None}

        def normalize_a(pvs):
            # ACT does these copies: it has a natural lull at the pair
            # boundary (the next pair's first exp waits on its QK anyway),
            # while DVE is busy with filler-chunk copies the PE waits on.
            # bf16 is enough: the values get rounded to bf16 at O anyway,
            # and the sums row feeds a bf16 ones-matmul broadcast (a bf16
            # denominator costs ~0.2% extra error, far within budget).
            ov = ovpool.tile([65, 2, 512], BF16, tag="ov", name="ov")
            for hi in (0, 1):
                nc.scalar.copy(ov[0:65, hi, :], pvs[hi][0:65, :])
            return ov

        def normalize_b(J, p, ov):
            brec = brpool.tile([64, 2, 512], F32, tag="brec", name="brec")
            for hi in (0, 1):
                bcm = mm_ps.tile([64, 512], F32, tag="ps", name="bcm")
                nc.tensor.matmul(bcm[:, :], lhsT=ones_sb[64:65, 0:64],
                                 rhs=ov[64:65, hi, :], start=True, stop=True)
                nc.vector.reciprocal_approx_fast(brec[:, hi, :], bcm[0:64, :])
            ot = opool.tile([128, 512], BF16, tag=f"o{p}", name=f"o{p}")
            O_tiles[(J, p)] = ot
            for hi in (0, 1):
                nc.gpsimd.tensor_mul(ot[64 * hi:64 * hi + 64, :],
                                     ov[0:64, hi, :], brec[:, hi, :])

        def flush_norm():
            if pending["fn"] is not None:
                pending["fn"]()
                pending["fn"] = None

        # ---- main loop: q-chunk-major attention with paced filler. PV runs
        # one k-tile behind QK so the PE never waits head-of-line on an exp.
        for J in range(NJ):
            if J >= 2:
                # O tiles are double-buffered per pair: proj of chunk J-2
                # must be emitted before normalize(J) reuses its slot
                drain_proj_thru(J - 2)
            nki = 4 * J + 4

            def emit_pv(pvs, ki, qc0, nw, pb, p, nki):
                force(bmark[("V", ki)])
                for hi in (0, 1):
                    nc.tensor.matmul(
                        pvs[hi][:, qc0:qc0 + nw],
                        lhsT=V_sb[ki][:, 2 * p + hi, :],
                        rhs=pb[:, hi, 0:nw],
                        start=(ki == 0), stop=(ki == nki - 1))
                mstate["cA"] += 2 * nw

            for p in range(NP):
                force(bmark[("K", p, J)])
                force(bmark[("Q", p, J)])
                # also force the NEXT pair's K/Q (and the V tiles between
                # them in the block) now, so their PSUM->SBUF copies are a
                # full pair old by the time that pair's QK LDWEIGHTS needs
                # them — otherwise the PE waits on the DVE queue
                if p + 1 < NP:
                    force(bmark[("Q", p + 1, J)])
                elif J + 1 < NJ:
                    force(bmark[("Q", 0, J + 1)])
                begin_pair()
                pvs = {}
                for hi in (0, 1):
                    pvs[hi] = mm_ps.tile([65, 512], F32, tag="pv", name="pv", bufs=2)
                pend = None
                for ki in range(nki):
                    e = ki - 4 * J
                    qc0 = 0 if e < 0 else 128 * e
                    nw = 512 - qc0
                    st = st_ps.tile([128, 2, 512], F32, tag="st", name="st")
                    for hi, h_off in ((0, 0), (1, 64)):
                        nc.tensor.matmul(
                            st[:, hi, 0:nw],
                            lhsT=K_sb[p][h_off:h_off + 64, 128 * ki:128 * ki + 128],
                            rhs=Q_sb[p][h_off:h_off + 64, 512 * J + qc0:512 * J + 512],
                            start=True, stop=True)
                    mstate["cA"] += nw
                    pb = ppool.tile([128, 2, 512], BF16, tag="pb", name="pb")
                    nc.scalar.activation(pb[:, :, 0:nw], st[:, :, 0:nw], EXP)
                    if e >= 0:
                        # multiplicative 0/1 causal stair on both heads; the
                        # mask operand repeats via a 0-stride dim
                        msrc = mask_sb[:, 0:128]
                        mrep = bass.AP(tensor=msrc.tensor, offset=msrc.offset,
                                       ap=[list(msrc.ap[0]), [0, 2], [1, 128]])
                        nc.vector.tensor_mul(pb[:, :, 0:128], pb[:, :, 0:128], mrep)
                    pace()
                    if ki == 1:
                        flush_norm()
                    if pend is not None:
                        emit_pv(pvs, *pend, p, nki)
                    pend = (ki, qc0, nw, pb)
                emit_pv(pvs, *pend, p, nki)
                ov = normalize_a(pvs)
                pending["fn"] = (lambda J=J, p=p, ov=ov: normalize_b(J, p, ov))
                pace()
            flush_norm()
            for m in range(8):
                pq.append((2048, (lambda J=J, m=m: emit_proj(J, m)), J))
            mstate["future_proj"] = 2048 * 8 * (NJ - 1 - J)

        drain_proj_thru(NJ)
        force(len(bq))

    nc.finalize()
    return nc


def _host_inputs(x, W_qkv, W_proj):
    bf = ml_dtypes.bfloat16
    kk, qq = np.meshgrid(np.arange(128), np.arange(128), indexing="ij")
    stair = np.ascontiguousarray((kk <= qq).astype(np.float32).astype(bf))

    in_maps = []
    for c in range(NCORES):
        b, g = c // 2, c % 2
        xT = np.ascontiguousarray(x[b].T.astype(bf))  # [D, T]
        s0 = 512 * g
        wq_ = np.ascontiguousarray((W_qkv[s0:s0 + 512] / 8.0).T.astype(bf))
        wk_ = np.ascontiguousarray(W_qkv[D + s0:D + s0 + 512].T.astype(bf))
        wv_ = np.ascontiguousarray(W_qkv[2 * D + s0:2 * D + s0 + 512].T.astype(bf))
        wp_ = np.ascontiguousarray(W_proj.T[s0:s0 + 512, :].astype(bf))
        in_maps.append({
            "xT": xT,
            "w_qT": wq_, "w_kT": wk_, "w_vT": wv_, "w_pT": wp_,
            "mask": stair,
        })
    return in_maps


def _run(inputs, trace=False, trace_cores=None):
    if "nc" not in _cached:
        _cached["nc"] = _build_program()
    nc = _cached["nc"]
    in_maps = _host_inputs(inputs["x"], inputs["W_qkv"], inputs["W_proj"])
    res = run_bass_kernel_spmd(
        nc, in_maps, core_ids=list(range(NCORES)),
        trace=trace, trace_cores=trace_cores,
    )
    out = np.zeros((B, T, D), np.float32)
    for b in range(B):
        acc = res.results[2 * b]["out_T"] + res.results[2 * b + 1]["out_T"]
        out[b] = acc.T  # [D, T] -> [T, D]
    return out, res


def kernel(**inputs) -> np.ndarray:
    out, _ = _run(inputs, trace=os.environ.get("KERNEL_TRACE", "") == "1")
    return out
